# revision 7
# baseline (speedup 1.0000x reference)
"""Trainium2 Bass kernel: batched multi-head attention with int mask.

Computes, per (b, h):
    S = (Q * D^-0.5) @ K^T
    P = exp(S) * mask          (mask in {0,1}; equals softmax numerator of
                                masked scores since exp(-inf) == 0)
    sums = P @ ones            (row sums over k)
    attn = P / sums            (== softmax(masked scores), 0 where masked)
    out  = (P @ V) / sums

Sharding: B*H = 64 (b, h) pairs split across 8 cores; each core owns one
batch and 8 heads, so each core loads only its batch's mask.

On-chip strategy (no big on-chip transposes; matmuls in float32r —
single-pass PE at bf16 speed, ~1.5e-4 relative precision):
  - Q^T, K^T shipped from host as [D, S] f32r (contraction on partitions).
  - Scores are computed twice, in both layouts, on the PE (recompute is
    cheaper than transposing P, which runs at LDWEIGHTS rate):
      path B: S^T tiles [k_part, q] -> exp -> *maskT -> P^T, feeds
              out^T = [V | 1]^T @ P^T  (ones column gives row sums free)
      path A: S tiles [q_part, k]   -> exp(s - ln sum) * maskN -> attn
  - The [65, q] out^T+sums block is PE-transposed per 128-q block, which
    lands sums on partitions: 1/sum and ln(1/sum) become per-partition
    scalars for the out normalize (tensor_scalar) and path A's exp bias.
    ln(1/sum) is batched per half (one ACT Ln) to avoid table thrash.
  - masks shipped from host as {0,1} fp8e4 in both layouts; path B's
    mask multiply runs on DVE, path A's on GpSimd (parallel engines).
  - V shipped pre-tiled [P, KT, D+1] f32r with the ones column appended.

Outputs: attn written in natural layout; out written in a permuted
[NH, P, NQB, D] layout (contiguous DMA) and un-permuted on host.
"""

import numpy as np
import ml_dtypes

import concourse.bacc as bacc
import concourse.tile as tile
from concourse import mybir
from concourse.bass_utils import run_bass_kernel_spmd
from concourse.masks import make_identity

# Problem shape (hardcoded; harness contract).
B, H, S, D = 4, 16, 2048, 64
N_CORES = 8
HPC = (B * H) // N_CORES  # heads per core = 8

P = 128
F32 = mybir.dt.float32
F32R = mybir.dt.float32r
FP8 = mybir.dt.float8e4
FP8NP = ml_dtypes.float8_e4m3
EXP = mybir.ActivationFunctionType.Exp
LN = mybir.ActivationFunctionType.Ln
MULT = mybir.AluOpType.mult


def build_nc(hpc=HPC, s=S, d=D, n_cores=N_CORES, qh=1024):
    KT = s // P              # k tiles
    qh = min(qh, s)
    NH = s // qh             # q chunks ("halves") per head
    NQB = qh // P            # q blocks per half
    QC = min(512, qh)        # matmul moving free dim (one PSUM bank)
    NQC = qh // QC

    nc = bacc.Bacc("TRN2", target_bir_lowering=False, debug=False,
                   num_devices=n_cores)

    qT_d = nc.dram_tensor("qT", [hpc, P, s], F32R, kind="ExternalInput").ap()
    kT_d = nc.dram_tensor("kT", [hpc, P, s], F32R, kind="ExternalInput").ap()
    v1_d = nc.dram_tensor("v1", [hpc, P, KT, d + 1], F32R,
                          kind="ExternalInput").ap()
    mN_d = nc.dram_tensor("maskN", [P, s // P, s], FP8,
                          kind="ExternalInput").ap()
    mT_d = nc.dram_tensor("maskT", [P, KT, s], FP8,
                          kind="ExternalInput").ap()
    out_d = nc.dram_tensor("out", [hpc, NH, P, NQB, d], F32,
                           kind="ExternalOutput").ap()
    attn_d = nc.dram_tensor("attn", [hpc, s, s], F32,
                            kind="ExternalOutput").ap()

    with tile.TileContext(nc) as tc:
        with tc.tile_pool(name="singles", bufs=1) as singles, \
             tc.tile_pool(name="qk", bufs=2) as qkp, \
             tc.tile_pool(name="pTp", bufs=1) as pTp, \
             tc.tile_pool(name="attnp", bufs=3) as attnp, \
             tc.tile_pool(name="posb", bufs=2) as posbp, \
             tc.tile_pool(name="outp", bufs=2) as outpp, \
             tc.tile_pool(name="smalls", bufs=8) as smalls, \
             tc.tile_pool(name="ps_a", bufs=2, space="PSUM") as ps_a, \
             tc.tile_pool(name="ps_o", bufs=1, space="PSUM") as ps_o, \
             tc.tile_pool(name="ps_t", bufs=2, space="PSUM") as ps_t:

            ident = singles.tile([P, P], F32)
            make_identity(nc, ident)
            maskN = singles.tile([P, s // P, s], FP8)
            nc.sync.dma_start(maskN, mN_d)
            maskT = singles.tile([P, KT, s], FP8)
            nc.sync.dma_start(maskT, mT_d)

            for h in range(hpc):
                qsb = qkp.tile([P, s], F32R, tag="qsb")
                nc.sync.dma_start(qsb, qT_d[h])
                ksb = qkp.tile([P, s], F32R, tag="ksb")
                nc.sync.dma_start(ksb, kT_d[h])
                v1 = qkp.tile([P, KT, d + 1], F32R, tag="v1")
                nc.sync.dma_start(v1, v1_d[h])

                for half in range(NH):
                    q0 = half * qh
                    # ---- path B: S^T -> P^T -------------------------------
                    # Contraction is only d=64, so pack two k-tiles into
                    # the PE as concurrent row-group matmuls (rows 0-63 /
                    # 64-127; Q^T,K^T are replicated across both halves).
                    pT = pTp.tile([P, KT, qh], F32R, tag="pT")
                    for tp in range(KT // 2):
                        tA, tB = 2 * tp, 2 * tp + 1
                        for c in range(NQC):
                            psS = ps_a.tile([P, 2 * QC], F32, tag="big")
                            qs = slice(q0 + c * QC, q0 + (c + 1) * QC)
                            nc.tensor.matmul(
                                psS[:, 0:QC],
                                lhsT=ksb[0:d, tA * P:(tA + 1) * P],
                                rhs=qsb[0:d, qs],
                                start=True, stop=True)
                            nc.tensor.matmul(
                                psS[:, QC:2 * QC],
                                lhsT=ksb[d:2 * d, tB * P:(tB + 1) * P],
                                rhs=qsb[d:2 * d, qs],
                                start=True, stop=True)
                            # psS = [S^T(tA) | S^T(tB)] for this q chunk
                            nc.scalar.activation(
                                out=pT[:, tA:tB + 1, c * QC:(c + 1) * QC],
                                in_=psS.rearrange("p (t q) -> p t q", t=2),
                                func=EXP)
                        nc.vector.tensor_tensor(
                            out=pT[:, tA:tB + 1, :],
                            in0=pT[:, tA:tB + 1, :],
                            in1=maskT[:, tA:tB + 1, q0:q0 + qh], op=MULT)
                    # ---- out^T and row sums: [V|1]^T @ P^T ----------------
                    po = ps_o.tile([d + 1, qh], F32, tag="po")
                    for c in range(NQC):
                        for t in range(KT):
                            nc.tensor.matmul(
                                po[:, c * QC:(c + 1) * QC],
                                lhsT=v1[:, t, :],
                                rhs=pT[:, t, c * QC:(c + 1) * QC],
                                start=(t == 0), stop=(t == KT - 1))
                    po_sb = posbp.tile([d + 1, qh], F32, tag="po_sb")
                    nc.vector.tensor_copy(out=po_sb, in_=po)

                    # transpose [65, 128] blocks: sums land on partitions
                    out_sb = outpp.tile([P, NQB, d], F32, tag="out_sb")
                    rec_all = smalls.tile([P, NQB], F32, tag="rec")
                    lnr_all = smalls.tile([P, NQB], F32, tag="lnr")
                    for qb in range(NQB):
                        psB = ps_t.tile([P, d + 1], F32, tag="pst")
                        nc.tensor.transpose(
                            psB, po_sb[:, qb * P:(qb + 1) * P],
                            ident[0:d + 1, 0:d + 1])
                        nc.vector.reciprocal(rec_all[:, qb:qb + 1],
                                             psB[:, d:d + 1])
                        nc.vector.tensor_scalar(
                            out_sb[:, qb, :], psB[:, 0:d],
                            rec_all[:, qb:qb + 1], None, MULT)
                    # ln(1/sum) for all blocks in one ACT op
                    nc.scalar.activation(out=lnr_all, in_=rec_all, func=LN)

                    # ---- path A: S tiles -> attn --------------------------
                    for qb in range(NQB):
                        qt = (q0 + qb * P) // P
                        attn_sb = attnp.tile([P, s], F32, tag="attn")
                        qs = slice(q0 + qb * P, q0 + (qb + 1) * P)
                        for half_k in range(s // qh):
                            k0 = half_k * qh
                            psA = ps_a.tile([P, qh], F32, tag="big")
                            # adjacent k chunks as concurrent row-group
                            # matmuls (d=64 contraction, rows 0-63 / 64-127)
                            for c in range(NQC):
                                lo = (c % 2) * d
                                nc.tensor.matmul(
                                    psA[:, c * QC:(c + 1) * QC],
                                    lhsT=qsb[lo:lo + d, qs],
                                    rhs=ksb[lo:lo + d,
                                            k0 + c * QC:k0 + (c + 1) * QC],
                                    start=True, stop=True)
                            # exp(s - ln sum) = exp(s)/sum
                            nc.scalar.activation(
                                out=attn_sb[:, k0:k0 + qh], in_=psA,
                                func=EXP, bias=lnr_all[:, qb:qb + 1])
                        nc.gpsimd.tensor_tensor(
                            out=attn_sb, in0=attn_sb, in1=maskN[:, qt, :],
                            op=MULT)
                        nc.sync.dma_start(
                            attn_d[h, q0 + qb * P: q0 + (qb + 1) * P, :],
                            attn_sb)
                    nc.sync.dma_start(out_d[h, half], out_sb)

    nc.compile()
    return nc


def prep_core_inputs(q, k, v, m01_N, m01_T, s=S, d=D):
    """Host-side shard prep for one core.

    q/k/v: [hpc, s, d] float32 for this core's heads.
    m01_N/m01_T: fp8 {0,1} masks in [P, s//P, s] layouts (shared per batch).
    """
    KT = s // P
    scale = np.float32(1.0 / np.sqrt(d))
    hpc = q.shape[0]
    # rows 0:d and d:2d hold the same data — feeds the two concurrent
    # row-group matmuls (base partitions 0 and 64)
    qT = np.empty((hpc, P, s), np.float32)
    qT[:, 0:d, :] = np.transpose(q, (0, 2, 1)) * scale
    qT[:, d:2 * d, :] = qT[:, 0:d, :]
    kT = np.empty((hpc, P, s), np.float32)
    kT[:, 0:d, :] = np.transpose(k, (0, 2, 1))
    kT[:, d:2 * d, :] = kT[:, 0:d, :]
    vt = v.reshape(q.shape[0], KT, P, d).transpose(0, 2, 1, 3)  # [hpc,P,KT,d]
    v1 = np.concatenate(
        [vt, np.ones(vt.shape[:-1] + (1,), np.float32)], axis=-1)
    return {
        "qT": qT,
        "kT": kT,
        "v1": np.ascontiguousarray(v1),
        "maskN": m01_N,
        "maskT": m01_T,
    }


def prep_masks(mask_b, s=S):
    """mask_b: [s, s] int -> ({0,1} fp8 [P, s//P, s] natural, transposed)."""
    m01 = (mask_b != 0).astype(np.float32)
    mN = np.ascontiguousarray(
        m01.reshape(s // P, P, s).transpose(1, 0, 2)).astype(FP8NP)
    mT = np.ascontiguousarray(
        m01.T.reshape(s // P, P, s).transpose(1, 0, 2)).astype(FP8NP)
    return mN, mT


_NC_CACHE = {}


def _get_nc():
    if "nc" not in _NC_CACHE:
        _NC_CACHE["nc"] = build_nc()
    return _NC_CACHE["nc"]


def kernel(query, key, value, mask):
    query = np.asarray(query, dtype=np.float32)
    key = np.asarray(key, dtype=np.float32)
    value = np.asarray(value, dtype=np.float32)
    mask = np.asarray(mask)

    nc = _get_nc()

    masks = [prep_masks(mask[b]) for b in range(B)]
    in_maps = []
    for c in range(N_CORES):
        b, hh = divmod(c, N_CORES // B)
        h0 = hh * HPC
        mN, mT = masks[b]
        in_maps.append(prep_core_inputs(
            query[b, h0:h0 + HPC], key[b, h0:h0 + HPC],
            value[b, h0:h0 + HPC], mN, mT))

    res = run_bass_kernel_spmd(nc, in_maps, list(range(N_CORES))).results

    out = np.empty((B, H, S, D), np.float32)
    attn = np.empty((B, H, S, S), np.float32)
    for c in range(N_CORES):
        b, hh = divmod(c, N_CORES // B)
        h0 = hh * HPC
        o = res[c]["out"]  # [HPC, NH, P, NQB, d]
        out[b, h0:h0 + HPC] = o.transpose(0, 1, 3, 2, 4).reshape(HPC, S, D)
        attn[b, h0:h0 + HPC] = res[c]["attn"]
    return out, attn


# revision 11
# speedup vs baseline: 1.0354x; 1.0354x over previous
"""Trainium2 Bass kernel: batched multi-head attention with int mask.

Computes, per (b, h):
    S = (Q * D^-0.5) @ K^T
    P = exp(S) * mask          (mask in {0,1}; equals softmax numerator of
                                masked scores since exp(-inf) == 0)
    sums = P @ ones            (row sums over k)
    attn = P / sums            (== softmax(masked scores), 0 where masked)
    out  = (P @ V) / sums

Sharding: B*H = 64 (b, h) pairs split across 8 cores; each core owns one
batch and 8 heads, so each core loads only its batch's mask.

On-chip strategy (no big on-chip transposes; matmuls in float32r —
single-pass PE at bf16 speed, ~1.5e-4 relative precision):
  - Q^T, K^T shipped from host as [D, S] f32r (contraction on partitions).
  - Scores are computed twice, in both layouts, on the PE (recompute is
    cheaper than transposing P, which runs at LDWEIGHTS rate):
      path B: S^T tiles [k_part, q] -> exp -> *maskT -> P^T, feeds
              out^T = [V | 1]^T @ P^T  (ones column gives row sums free)
      path A: S tiles [q_part, k]   -> exp(s - ln sum) * maskN -> attn
  - The [65, q] out^T+sums block is PE-transposed per 128-q block, which
    lands sums on partitions: 1/sum and ln(1/sum) become per-partition
    scalars for the out normalize (tensor_scalar) and path A's exp bias.
    ln(1/sum) is batched per half (one ACT Ln) to avoid table thrash.
  - masks shipped from host as {0,1} fp8e4 in both layouts; path B's
    mask multiply runs on DVE, path A's on GpSimd (parallel engines).
  - V shipped pre-tiled [P, KT, D+1] f32r with the ones column appended.

Outputs: attn written in natural layout; out written in a permuted
[NH, P, NQB, D] layout (contiguous DMA) and un-permuted on host.
"""

import numpy as np
import ml_dtypes

import concourse.bacc as bacc
import concourse.tile as tile
from concourse import mybir
from concourse.bass_utils import run_bass_kernel_spmd
from concourse.masks import make_identity

# Problem shape (hardcoded; harness contract).
B, H, S, D = 4, 16, 2048, 64
N_CORES = 8
HPC = (B * H) // N_CORES  # heads per core = 8

P = 128
F32 = mybir.dt.float32
F32R = mybir.dt.float32r
FP8 = mybir.dt.float8e4
FP8NP = ml_dtypes.float8_e4m3
EXP = mybir.ActivationFunctionType.Exp
LN = mybir.ActivationFunctionType.Ln
MULT = mybir.AluOpType.mult


def build_nc(hpc=HPC, s=S, d=D, n_cores=N_CORES, qh=1024):
    KT = s // P              # k tiles
    qh = min(qh, s)
    NH = s // qh             # q chunks ("halves") per head
    NQB = qh // P            # q blocks per half
    QC = min(512, qh)        # matmul moving free dim (one PSUM bank)
    NQC = qh // QC

    nc = bacc.Bacc("TRN2", target_bir_lowering=False, debug=False,
                   num_devices=n_cores)

    qT_d = nc.dram_tensor("qT", [hpc, P, s], F32R, kind="ExternalInput").ap()
    kT_d = nc.dram_tensor("kT", [hpc, P, s], F32R, kind="ExternalInput").ap()
    v1_d = nc.dram_tensor("v1", [hpc, P, KT, d + 1], F32R,
                          kind="ExternalInput").ap()
    mN_d = nc.dram_tensor("maskN", [P, s // P, s], FP8,
                          kind="ExternalInput").ap()
    mT_d = nc.dram_tensor("maskT", [P, KT, s], FP8,
                          kind="ExternalInput").ap()
    out_d = nc.dram_tensor("out", [hpc, NH, P, NQB, d], F32,
                           kind="ExternalOutput").ap()
    attn_d = nc.dram_tensor("attn", [hpc, s, s], F32,
                            kind="ExternalOutput").ap()

    with tile.TileContext(nc) as tc:
        with tc.tile_pool(name="singles", bufs=1) as singles, \
             tc.tile_pool(name="qk", bufs=2) as qkp, \
             tc.tile_pool(name="pTp", bufs=1) as pTp, \
             tc.tile_pool(name="attnp", bufs=3) as attnp, \
             tc.tile_pool(name="posb", bufs=1) as posbp, \
             tc.tile_pool(name="outp", bufs=2) as outpp, \
             tc.tile_pool(name="smalls", bufs=8) as smalls, \
             tc.tile_pool(name="ps_a", bufs=2, space="PSUM") as ps_a, \
             tc.tile_pool(name="ps_o", bufs=1, space="PSUM") as ps_o, \
             tc.tile_pool(name="ps_t", bufs=2, space="PSUM") as ps_t:

            ident = singles.tile([P, P], F32)
            make_identity(nc, ident)
            maskN = singles.tile([P, s // P, s], FP8)
            nc.sync.dma_start(maskN, mN_d)
            maskT = singles.tile([P, KT, s], FP8)
            nc.sync.dma_start(maskT, mT_d)

            for h in range(hpc):
                qsb = qkp.tile([P, s], F32R, tag="qsb")
                nc.sync.dma_start(qsb, qT_d[h])
                ksb = qkp.tile([P, s], F32R, tag="ksb")
                nc.sync.dma_start(ksb, kT_d[h])
                v1 = qkp.tile([P, KT, d + 1], F32R, tag="v1")
                nc.sync.dma_start(v1, v1_d[h])

                for half in range(NH):
                    q0 = half * qh
                    # ---- path B: S^T -> P^T -------------------------------
                    # Contraction is only d=64, so pack two k-tiles into
                    # the PE as concurrent row-group matmuls (rows 0-63 /
                    # 64-127; Q^T,K^T are replicated across both halves).
                    pT = pTp.tile([P, KT, qh], F32R, tag="pT")
                    # out^T accumulator (ones column of v1 gives row sums);
                    # its matmuls are interleaved into the k-tile loop so
                    # PE never waits for the full P^T.
                    po = ps_o.tile([d + 1, qh], F32, tag="po")
                    for tp in range(KT // 2):
                        tA, tB = 2 * tp, 2 * tp + 1
                        for c in range(NQC):
                            psS = ps_a.tile([P, 2 * QC], F32, tag="big")
                            qs = slice(q0 + c * QC, q0 + (c + 1) * QC)
                            nc.tensor.matmul(
                                psS[:, 0:QC],
                                lhsT=ksb[0:d, tA * P:(tA + 1) * P],
                                rhs=qsb[0:d, qs],
                                start=True, stop=True)
                            nc.tensor.matmul(
                                psS[:, QC:2 * QC],
                                lhsT=ksb[d:2 * d, tB * P:(tB + 1) * P],
                                rhs=qsb[d:2 * d, qs],
                                start=True, stop=True)
                            # psS = [S^T(tA) | S^T(tB)] for this q chunk
                            nc.scalar.activation(
                                out=pT[:, tA:tB + 1, c * QC:(c + 1) * QC],
                                in_=psS.rearrange("p (t q) -> p t q", t=2),
                                func=EXP)
                        nc.vector.tensor_tensor(
                            out=pT[:, tA:tB + 1, :],
                            in0=pT[:, tA:tB + 1, :],
                            in1=maskT[:, tA:tB + 1, q0:q0 + qh], op=MULT)
                        for t in (tA, tB):
                            for c in range(NQC):
                                nc.tensor.matmul(
                                    po[:, c * QC:(c + 1) * QC],
                                    lhsT=v1[:, t, :],
                                    rhs=pT[:, t, c * QC:(c + 1) * QC],
                                    start=(t == 0), stop=(t == KT - 1))
                    po_sb = posbp.tile([d + 1, qh], F32, tag="po_sb")
                    nc.vector.tensor_copy(out=po_sb, in_=po)

                    # transpose [65, 128] blocks: sums land on partitions
                    out_sb = outpp.tile([P, NQB, d], F32, tag="out_sb")
                    rec_all = smalls.tile([P, NQB], F32, tag="rec")
                    lnr_all = smalls.tile([P, NQB], F32, tag="lnr")
                    for qb in range(NQB):
                        psB = ps_t.tile([P, d + 1], F32, tag="pst")
                        nc.tensor.transpose(
                            psB, po_sb[:, qb * P:(qb + 1) * P],
                            ident[0:d + 1, 0:d + 1])
                        nc.vector.reciprocal(rec_all[:, qb:qb + 1],
                                             psB[:, d:d + 1])
                        nc.vector.tensor_scalar(
                            out_sb[:, qb, :], psB[:, 0:d],
                            rec_all[:, qb:qb + 1], None, MULT)
                    # ln(1/sum) for all blocks in one ACT op
                    nc.scalar.activation(out=lnr_all, in_=rec_all, func=LN)

                    # ---- path A: S tiles -> attn --------------------------
                    for qb in range(NQB):
                        qt = (q0 + qb * P) // P
                        attn_sb = attnp.tile([P, s], F32, tag="attn")
                        qs = slice(q0 + qb * P, q0 + (qb + 1) * P)
                        for half_k in range(s // qh):
                            k0 = half_k * qh
                            psA = ps_a.tile([P, qh], F32, tag="big")
                            # adjacent k chunks as concurrent row-group
                            # matmuls (d=64 contraction, rows 0-63 / 64-127)
                            for c in range(NQC):
                                lo = (c % 2) * d
                                nc.tensor.matmul(
                                    psA[:, c * QC:(c + 1) * QC],
                                    lhsT=qsb[lo:lo + d, qs],
                                    rhs=ksb[lo:lo + d,
                                            k0 + c * QC:k0 + (c + 1) * QC],
                                    start=True, stop=True)
                            # exp(s - ln sum) = exp(s)/sum
                            nc.scalar.activation(
                                out=attn_sb[:, k0:k0 + qh], in_=psA,
                                func=EXP, bias=lnr_all[:, qb:qb + 1])
                        nc.gpsimd.tensor_tensor(
                            out=attn_sb, in0=attn_sb, in1=maskN[:, qt, :],
                            op=MULT)
                        nc.sync.dma_start(
                            attn_d[h, q0 + qb * P: q0 + (qb + 1) * P, :],
                            attn_sb)
                    nc.sync.dma_start(out_d[h, half], out_sb)

    nc.compile()
    return nc


def prep_core_inputs(q, k, v, m01_N, m01_T, s=S, d=D):
    """Host-side shard prep for one core.

    q/k/v: [hpc, s, d] float32 for this core's heads.
    m01_N/m01_T: fp8 {0,1} masks in [P, s//P, s] layouts (shared per batch).
    """
    KT = s // P
    scale = np.float32(1.0 / np.sqrt(d))
    hpc = q.shape[0]
    # rows 0:d and d:2d hold the same data — feeds the two concurrent
    # row-group matmuls (base partitions 0 and 64)
    qT = np.empty((hpc, P, s), np.float32)
    qT[:, 0:d, :] = np.transpose(q, (0, 2, 1)) * scale
    qT[:, d:2 * d, :] = qT[:, 0:d, :]
    kT = np.empty((hpc, P, s), np.float32)
    kT[:, 0:d, :] = np.transpose(k, (0, 2, 1))
    kT[:, d:2 * d, :] = kT[:, 0:d, :]
    vt = v.reshape(q.shape[0], KT, P, d).transpose(0, 2, 1, 3)  # [hpc,P,KT,d]
    v1 = np.concatenate(
        [vt, np.ones(vt.shape[:-1] + (1,), np.float32)], axis=-1)
    return {
        "qT": qT,
        "kT": kT,
        "v1": np.ascontiguousarray(v1),
        "maskN": m01_N,
        "maskT": m01_T,
    }


def prep_masks(mask_b, s=S):
    """mask_b: [s, s] int -> ({0,1} fp8 [P, s//P, s] natural, transposed)."""
    m01 = (mask_b != 0).astype(np.float32)
    mN = np.ascontiguousarray(
        m01.reshape(s // P, P, s).transpose(1, 0, 2)).astype(FP8NP)
    mT = np.ascontiguousarray(
        m01.T.reshape(s // P, P, s).transpose(1, 0, 2)).astype(FP8NP)
    return mN, mT


_NC_CACHE = {}


def _get_nc():
    if "nc" not in _NC_CACHE:
        _NC_CACHE["nc"] = build_nc()
    return _NC_CACHE["nc"]


def kernel(query, key, value, mask):
    query = np.asarray(query, dtype=np.float32)
    key = np.asarray(key, dtype=np.float32)
    value = np.asarray(value, dtype=np.float32)
    mask = np.asarray(mask)

    nc = _get_nc()

    masks = [prep_masks(mask[b]) for b in range(B)]
    in_maps = []
    for c in range(N_CORES):
        b, hh = divmod(c, N_CORES // B)
        h0 = hh * HPC
        mN, mT = masks[b]
        in_maps.append(prep_core_inputs(
            query[b, h0:h0 + HPC], key[b, h0:h0 + HPC],
            value[b, h0:h0 + HPC], mN, mT))

    res = run_bass_kernel_spmd(nc, in_maps, list(range(N_CORES))).results

    out = np.empty((B, H, S, D), np.float32)
    attn = np.empty((B, H, S, S), np.float32)
    for c in range(N_CORES):
        b, hh = divmod(c, N_CORES // B)
        h0 = hh * HPC
        o = res[c]["out"]  # [HPC, NH, P, NQB, d]
        out[b, h0:h0 + HPC] = o.transpose(0, 1, 3, 2, 4).reshape(HPC, S, D)
        attn[b, h0:h0 + HPC] = res[c]["attn"]
    return out, attn


# revision 13
# speedup vs baseline: 1.2016x; 1.1605x over previous
"""Trainium2 Bass kernel: batched multi-head attention with int mask.

Computes, per (b, h):
    S = (Q * D^-0.5) @ K^T
    P = exp(S) * mask          (mask in {0,1}; equals softmax numerator of
                                masked scores since exp(-inf) == 0)
    sums = P @ ones            (row sums over k)
    attn = P / sums            (== softmax(masked scores), 0 where masked)
    out  = (P @ V) / sums

Sharding: B*H = 64 (b, h) pairs split across 8 cores; each core owns one
batch and 8 heads, so each core loads only its batch's mask.

On-chip strategy (no big on-chip transposes; matmuls in float32r —
single-pass PE at bf16 speed, ~1.5e-4 relative precision):
  - Q^T, K^T shipped from host as [D, S] f32r (contraction on partitions).
  - Scores are computed twice, in both layouts, on the PE (recompute is
    cheaper than transposing P, which runs at LDWEIGHTS rate):
      path B: S^T tiles [k_part, q] -> exp -> *maskT -> P^T, feeds
              out^T = [V | 1]^T @ P^T  (ones column gives row sums free)
      path A: S tiles [q_part, k]   -> exp(s - ln sum) * maskN -> attn
  - The [65, q] out^T+sums block is PE-transposed per 128-q block, which
    lands sums on partitions: 1/sum and ln(1/sum) become per-partition
    scalars for the out normalize (tensor_scalar) and path A's exp bias.
    ln(1/sum) is batched per half (one ACT Ln) to avoid table thrash.
  - masks shipped from host as {0,1} fp8e4 in both layouts; path B's
    mask multiply runs on DVE, path A's on GpSimd (parallel engines).
  - V shipped pre-tiled [P, KT, D+1] f32r with the ones column appended.

Outputs: attn written in natural layout; out written in a permuted
[NH, P, NQB, D] layout (contiguous DMA) and un-permuted on host.
"""

import numpy as np
import ml_dtypes

import concourse.bacc as bacc
import concourse.tile as tile
from concourse import mybir
from concourse.bass_utils import run_bass_kernel_spmd
from concourse.masks import make_identity

# Problem shape (hardcoded; harness contract).
B, H, S, D = 4, 16, 2048, 64
N_CORES = 8
HPC = (B * H) // N_CORES  # heads per core = 8

P = 128
F32 = mybir.dt.float32
F32R = mybir.dt.float32r
FP8 = mybir.dt.float8e4
FP8NP = ml_dtypes.float8_e4m3
EXP = mybir.ActivationFunctionType.Exp
LN = mybir.ActivationFunctionType.Ln
MULT = mybir.AluOpType.mult


def build_nc(hpc=HPC, s=S, d=D, n_cores=N_CORES, qh=1024):
    KT = s // P              # k tiles
    qh = min(qh, s)
    NH = s // qh             # q chunks ("halves") per head
    NQB = qh // P            # q blocks per half
    QC = min(512, qh)        # matmul moving free dim (one PSUM bank)
    NQC = qh // QC

    nc = bacc.Bacc("TRN2", target_bir_lowering=False, debug=False,
                   num_devices=n_cores)

    qT_d = nc.dram_tensor("qT", [hpc, P, s], F32R, kind="ExternalInput").ap()
    kT_d = nc.dram_tensor("kT", [hpc, P, s], F32R, kind="ExternalInput").ap()
    v1_d = nc.dram_tensor("v1", [hpc, P, KT, d + 1], F32R,
                          kind="ExternalInput").ap()
    mN_d = nc.dram_tensor("maskN", [P, s // P, s], FP8,
                          kind="ExternalInput").ap()
    mT_d = nc.dram_tensor("maskT", [P, KT, s], FP8,
                          kind="ExternalInput").ap()
    out_d = nc.dram_tensor("out", [hpc, NH, P, NQB, d], F32,
                           kind="ExternalOutput").ap()
    attn_d = nc.dram_tensor("attn", [hpc, s, s], F32,
                            kind="ExternalOutput").ap()

    with tile.TileContext(nc) as tc:
        with tc.tile_pool(name="singles", bufs=1) as singles, \
             tc.tile_pool(name="qk", bufs=2) as qkp, \
             tc.tile_pool(name="pTp", bufs=1) as pTp, \
             tc.tile_pool(name="attnp", bufs=3) as attnp, \
             tc.tile_pool(name="posb", bufs=1) as posbp, \
             tc.tile_pool(name="outp", bufs=2) as outpp, \
             tc.tile_pool(name="smalls", bufs=8) as smalls, \
             tc.tile_pool(name="ps_a", bufs=2, space="PSUM") as ps_a, \
             tc.tile_pool(name="ps_b", bufs=1, space="PSUM") as ps_b, \
             tc.tile_pool(name="ps_o", bufs=1, space="PSUM") as ps_o:

            ident = singles.tile([P, P], F32)
            make_identity(nc, ident)
            maskN = singles.tile([P, s // P, s], FP8)
            nc.sync.dma_start(maskN, mN_d)
            maskT = singles.tile([P, KT, s], FP8)
            nc.sync.dma_start(maskT, mT_d)

            for h in range(hpc):
                qsb = qkp.tile([P, s], F32R, tag="qsb")
                nc.sync.dma_start(qsb, qT_d[h])
                ksb = qkp.tile([P, s], F32R, tag="ksb")
                nc.sync.dma_start(ksb, kT_d[h])
                v1 = qkp.tile([P, KT, d + 1], F32R, tag="v1")
                nc.sync.dma_start(v1, v1_d[h])

                for half in range(NH):
                    q0 = half * qh
                    # ---- path B: S^T -> P^T -------------------------------
                    # Contraction is only d=64, so pack two k-tiles into
                    # the PE as concurrent row-group matmuls (rows 0-63 /
                    # 64-127; Q^T,K^T are replicated across both halves).
                    pT = pTp.tile([P, KT, qh], F32R, tag="pT")
                    # out^T accumulator (ones column of v1 gives row sums);
                    # its matmuls are interleaved into the k-tile loop so
                    # PE never waits for the full P^T.
                    po = ps_o.tile([d + 1, qh], F32, tag="po")
                    for tp in range(KT // 2):
                        tA, tB = 2 * tp, 2 * tp + 1
                        for c in range(NQC):
                            psS = ps_a.tile([P, 2 * QC], F32, tag="big")
                            qs = slice(q0 + c * QC, q0 + (c + 1) * QC)
                            nc.tensor.matmul(
                                psS[:, 0:QC],
                                lhsT=ksb[0:d, tA * P:(tA + 1) * P],
                                rhs=qsb[0:d, qs],
                                start=True, stop=True)
                            nc.tensor.matmul(
                                psS[:, QC:2 * QC],
                                lhsT=ksb[d:2 * d, tB * P:(tB + 1) * P],
                                rhs=qsb[d:2 * d, qs],
                                start=True, stop=True)
                            # psS = [S^T(tA) | S^T(tB)] for this q chunk
                            nc.scalar.activation(
                                out=pT[:, tA:tB + 1, c * QC:(c + 1) * QC],
                                in_=psS.rearrange("p (t q) -> p t q", t=2),
                                func=EXP)
                        nc.vector.tensor_tensor(
                            out=pT[:, tA:tB + 1, :],
                            in0=pT[:, tA:tB + 1, :],
                            in1=maskT[:, tA:tB + 1, q0:q0 + qh], op=MULT)
                        for t in (tA, tB):
                            for c in range(NQC):
                                nc.tensor.matmul(
                                    po[:, c * QC:(c + 1) * QC],
                                    lhsT=v1[:, t, :],
                                    rhs=pT[:, t, c * QC:(c + 1) * QC],
                                    start=(t == 0), stop=(t == KT - 1))
                    po_sb = posbp.tile([d + 1, qh], F32, tag="po_sb")
                    nc.vector.tensor_copy(out=po_sb, in_=po)

                    # transpose [65, 128] blocks: sums land on partitions
                    out_sb = outpp.tile([P, NQB, d], F32, tag="out_sb")
                    rec_all = smalls.tile([P, NQB], F32, tag="rec")
                    lnr_all = smalls.tile([P, NQB], F32, tag="lnr")
                    for qb in range(NQB):
                        psB = ps_o.tile([P, d + 1], F32, tag="po")
                        nc.tensor.transpose(
                            psB, po_sb[:, qb * P:(qb + 1) * P],
                            ident[0:d + 1, 0:d + 1])
                        nc.vector.reciprocal(rec_all[:, qb:qb + 1],
                                             psB[:, d:d + 1])
                        nc.vector.tensor_scalar(
                            out_sb[:, qb, :], psB[:, 0:d],
                            rec_all[:, qb:qb + 1], None, MULT)
                    # ln(1/sum) for all blocks in one ACT op
                    nc.scalar.activation(out=lnr_all, in_=rec_all, func=LN)

                    # ---- path A: S tiles -> attn --------------------------
                    for qb in range(NQB):
                        qt = (q0 + qb * P) // P
                        attn_sb = attnp.tile([P, s], F32, tag="attn")
                        qs = slice(q0 + qb * P, q0 + (qb + 1) * P)
                        for half_k in range(s // qh):
                            k0 = half_k * qh
                            psA = ps_b.tile([P, qh], F32, tag="bigA")
                            # adjacent k chunks as concurrent row-group
                            # matmuls (d=64 contraction, rows 0-63 / 64-127)
                            for c in range(NQC):
                                lo = (c % 2) * d
                                nc.tensor.matmul(
                                    psA[:, c * QC:(c + 1) * QC],
                                    lhsT=qsb[lo:lo + d, qs],
                                    rhs=ksb[lo:lo + d,
                                            k0 + c * QC:k0 + (c + 1) * QC],
                                    start=True, stop=True)
                            # exp(s - ln sum) = exp(s)/sum
                            nc.scalar.activation(
                                out=attn_sb[:, k0:k0 + qh], in_=psA,
                                func=EXP, bias=lnr_all[:, qb:qb + 1])
                        meng = nc.vector if qb % 4 == 3 else nc.gpsimd
                        meng.tensor_tensor(
                            out=attn_sb, in0=attn_sb, in1=maskN[:, qt, :],
                            op=MULT)
                        nc.sync.dma_start(
                            attn_d[h, q0 + qb * P: q0 + (qb + 1) * P, :],
                            attn_sb)
                    nc.sync.dma_start(out_d[h, half], out_sb)

    nc.compile()
    return nc


def prep_core_inputs(q, k, v, m01_N, m01_T, s=S, d=D):
    """Host-side shard prep for one core.

    q/k/v: [hpc, s, d] float32 for this core's heads.
    m01_N/m01_T: fp8 {0,1} masks in [P, s//P, s] layouts (shared per batch).
    """
    KT = s // P
    scale = np.float32(1.0 / np.sqrt(d))
    hpc = q.shape[0]
    # rows 0:d and d:2d hold the same data — feeds the two concurrent
    # row-group matmuls (base partitions 0 and 64)
    qT = np.empty((hpc, P, s), np.float32)
    qT[:, 0:d, :] = np.transpose(q, (0, 2, 1)) * scale
    qT[:, d:2 * d, :] = qT[:, 0:d, :]
    kT = np.empty((hpc, P, s), np.float32)
    kT[:, 0:d, :] = np.transpose(k, (0, 2, 1))
    kT[:, d:2 * d, :] = kT[:, 0:d, :]
    vt = v.reshape(q.shape[0], KT, P, d).transpose(0, 2, 1, 3)  # [hpc,P,KT,d]
    v1 = np.concatenate(
        [vt, np.ones(vt.shape[:-1] + (1,), np.float32)], axis=-1)
    return {
        "qT": qT,
        "kT": kT,
        "v1": np.ascontiguousarray(v1),
        "maskN": m01_N,
        "maskT": m01_T,
    }


def prep_masks(mask_b, s=S):
    """mask_b: [s, s] int -> ({0,1} fp8 [P, s//P, s] natural, transposed)."""
    m01 = (mask_b != 0).astype(np.float32)
    mN = np.ascontiguousarray(
        m01.reshape(s // P, P, s).transpose(1, 0, 2)).astype(FP8NP)
    mT = np.ascontiguousarray(
        m01.T.reshape(s // P, P, s).transpose(1, 0, 2)).astype(FP8NP)
    return mN, mT


_NC_CACHE = {}


def _get_nc():
    if "nc" not in _NC_CACHE:
        _NC_CACHE["nc"] = build_nc()
    return _NC_CACHE["nc"]


def kernel(query, key, value, mask):
    query = np.asarray(query, dtype=np.float32)
    key = np.asarray(key, dtype=np.float32)
    value = np.asarray(value, dtype=np.float32)
    mask = np.asarray(mask)

    nc = _get_nc()

    masks = [prep_masks(mask[b]) for b in range(B)]
    in_maps = []
    for c in range(N_CORES):
        b, hh = divmod(c, N_CORES // B)
        h0 = hh * HPC
        mN, mT = masks[b]
        in_maps.append(prep_core_inputs(
            query[b, h0:h0 + HPC], key[b, h0:h0 + HPC],
            value[b, h0:h0 + HPC], mN, mT))

    res = run_bass_kernel_spmd(nc, in_maps, list(range(N_CORES))).results

    out = np.empty((B, H, S, D), np.float32)
    attn = np.empty((B, H, S, S), np.float32)
    for c in range(N_CORES):
        b, hh = divmod(c, N_CORES // B)
        h0 = hh * HPC
        o = res[c]["out"]  # [HPC, NH, P, NQB, d]
        out[b, h0:h0 + HPC] = o.transpose(0, 1, 3, 2, 4).reshape(HPC, S, D)
        attn[b, h0:h0 + HPC] = res[c]["attn"]
    return out, attn


# revision 15
# speedup vs baseline: 1.2036x; 1.0017x over previous
"""Trainium2 Bass kernel: batched multi-head attention with int mask.

Computes, per (b, h):
    S = (Q * D^-0.5) @ K^T
    P = exp(S) * mask          (mask in {0,1}; equals softmax numerator of
                                masked scores since exp(-inf) == 0)
    sums = P @ ones            (row sums over k)
    attn = P / sums            (== softmax(masked scores), 0 where masked)
    out  = (P @ V) / sums

Sharding: B*H = 64 (b, h) pairs split across 8 cores; each core owns one
batch and 8 heads, so each core loads only its batch's mask.

On-chip strategy (no big on-chip transposes; matmuls in float32r —
single-pass PE at bf16 speed, ~1.5e-4 relative precision):
  - Q^T, K^T shipped from host as [D, S] f32r (contraction on partitions).
  - Scores are computed twice, in both layouts, on the PE (recompute is
    cheaper than transposing P, which runs at LDWEIGHTS rate):
      path B: S^T tiles [k_part, q] -> exp -> *maskT -> P^T, feeds
              out^T = [V | 1]^T @ P^T  (ones column gives row sums free)
      path A: S tiles [q_part, k]   -> exp(s - ln sum) * maskN -> attn
  - The [65, q] out^T+sums block is PE-transposed per 128-q block, which
    lands sums on partitions: 1/sum and ln(1/sum) become per-partition
    scalars for the out normalize (tensor_scalar) and path A's exp bias.
    ln(1/sum) is batched per half (one ACT Ln) to avoid table thrash.
  - masks shipped from host as {0,1} fp8e4 in both layouts; path B's
    mask multiply runs on DVE, path A's on GpSimd (parallel engines).
  - V shipped pre-tiled [P, KT, D+1] f32r with the ones column appended.

Outputs: attn written in natural layout; out written in a permuted
[NH, P, NQB, D] layout (contiguous DMA) and un-permuted on host.
"""

import numpy as np
import ml_dtypes

import concourse.bacc as bacc
import concourse.tile as tile
from concourse import mybir
from concourse.bass_utils import run_bass_kernel_spmd
from concourse.masks import make_identity

# Problem shape (hardcoded; harness contract).
B, H, S, D = 4, 16, 2048, 64
N_CORES = 8
HPC = (B * H) // N_CORES  # heads per core = 8

P = 128
F32 = mybir.dt.float32
F32R = mybir.dt.float32r
FP8 = mybir.dt.float8e4
FP8NP = ml_dtypes.float8_e4m3
EXP = mybir.ActivationFunctionType.Exp
LN = mybir.ActivationFunctionType.Ln
MULT = mybir.AluOpType.mult


def build_nc(hpc=HPC, s=S, d=D, n_cores=N_CORES, qh=1024):
    KT = s // P              # k tiles
    qh = min(qh, s)
    NH = s // qh             # q chunks ("halves") per head
    NQB = qh // P            # q blocks per half
    QC = min(512, qh)        # matmul moving free dim (one PSUM bank)
    NQC = qh // QC

    nc = bacc.Bacc("TRN2", target_bir_lowering=False, debug=False,
                   num_devices=n_cores)

    qT_d = nc.dram_tensor("qT", [hpc, P, s], F32R, kind="ExternalInput").ap()
    kT_d = nc.dram_tensor("kT", [hpc, P, s], F32R, kind="ExternalInput").ap()
    v1_d = nc.dram_tensor("v1", [hpc, P, KT, d + 1], F32R,
                          kind="ExternalInput").ap()
    mN_d = nc.dram_tensor("maskN", [P, s // P, s], FP8,
                          kind="ExternalInput").ap()
    mT_d = nc.dram_tensor("maskT", [P, KT, s], FP8,
                          kind="ExternalInput").ap()
    out_d = nc.dram_tensor("out", [hpc, NH, P, NQB, d], F32,
                           kind="ExternalOutput").ap()
    attn_d = nc.dram_tensor("attn", [hpc, s, s], F32,
                            kind="ExternalOutput").ap()

    with tile.TileContext(nc) as tc:
        with tc.tile_pool(name="singles", bufs=1) as singles, \
             tc.tile_pool(name="qk", bufs=2) as qkp, \
             tc.tile_pool(name="pTp", bufs=1) as pTp, \
             tc.tile_pool(name="attnp", bufs=3) as attnp, \
             tc.tile_pool(name="posb", bufs=1) as posbp, \
             tc.tile_pool(name="outp", bufs=2) as outpp, \
             tc.tile_pool(name="smalls", bufs=8) as smalls, \
             tc.tile_pool(name="ps_a", bufs=2, space="PSUM") as ps_a, \
             tc.tile_pool(name="ps_b", bufs=1, space="PSUM") as ps_b, \
             tc.tile_pool(name="ps_o", bufs=1, space="PSUM") as ps_o:

            ident = singles.tile([P, P], F32)
            make_identity(nc, ident)
            maskN = singles.tile([P, s // P, s], FP8)
            nc.sync.dma_start(maskN, mN_d)
            maskT = singles.tile([P, KT, s], FP8)
            nc.sync.dma_start(maskT, mT_d)

            for h in range(hpc):
                qsb = qkp.tile([P, s], F32R, tag="qsb")
                nc.sync.dma_start(qsb, qT_d[h])
                ksb = qkp.tile([P, s], F32R, tag="ksb")
                nc.sync.dma_start(ksb, kT_d[h])
                v1 = qkp.tile([P, KT, d + 1], F32R, tag="v1")
                nc.sync.dma_start(v1, v1_d[h])

                for half in range(NH):
                    q0 = half * qh
                    # ---- path B: S^T -> P^T -------------------------------
                    # Contraction is only d=64, so pack two k-tiles into
                    # the PE as concurrent row-group matmuls (rows 0-63 /
                    # 64-127; Q^T,K^T are replicated across both halves).
                    pT = pTp.tile([P, KT, qh], F32R, tag="pT")
                    # out^T accumulator (ones column of v1 gives row sums);
                    # its matmuls are interleaved into the k-tile loop so
                    # PE never waits for the full P^T.
                    po = ps_o.tile([d + 1, qh], F32, tag="po")
                    for tp in range(KT // 2):
                        tA, tB = 2 * tp, 2 * tp + 1
                        for c in range(NQC):
                            psS = ps_a.tile([P, 2 * QC], F32, tag="big")
                            qs = slice(q0 + c * QC, q0 + (c + 1) * QC)
                            nc.tensor.matmul(
                                psS[:, 0:QC],
                                lhsT=ksb[0:d, tA * P:(tA + 1) * P],
                                rhs=qsb[0:d, qs],
                                start=True, stop=True)
                            nc.tensor.matmul(
                                psS[:, QC:2 * QC],
                                lhsT=ksb[d:2 * d, tB * P:(tB + 1) * P],
                                rhs=qsb[d:2 * d, qs],
                                start=True, stop=True)
                            # psS = [S^T(tA) | S^T(tB)] for this q chunk
                            nc.scalar.activation(
                                out=pT[:, tA:tB + 1, c * QC:(c + 1) * QC],
                                in_=psS.rearrange("p (t q) -> p t q", t=2),
                                func=EXP)
                        nc.vector.tensor_tensor(
                            out=pT[:, tA:tB + 1, :],
                            in0=pT[:, tA:tB + 1, :],
                            in1=maskT[:, tA:tB + 1, q0:q0 + qh], op=MULT)
                        for t in (tA, tB):
                            for c in range(NQC):
                                nc.tensor.matmul(
                                    po[:, c * QC:(c + 1) * QC],
                                    lhsT=v1[:, t, :],
                                    rhs=pT[:, t, c * QC:(c + 1) * QC],
                                    start=(t == 0), stop=(t == KT - 1))
                    po_sb = posbp.tile([d + 1, qh], F32, tag="po_sb")
                    nc.vector.tensor_copy(out=po_sb, in_=po)

                    # transpose [65, 128] blocks: sums land on partitions
                    out_sb = outpp.tile([P, NQB, d], F32, tag="out_sb")
                    rec_all = smalls.tile([P, NQB], F32, tag="rec")
                    lnr_all = smalls.tile([P, NQB], F32, tag="lnr")
                    for qb in range(NQB):
                        psB = ps_o.tile([P, d + 1], F32, tag="po")
                        nc.tensor.transpose(
                            psB, po_sb[:, qb * P:(qb + 1) * P],
                            ident[0:d + 1, 0:d + 1])
                        nc.vector.reciprocal(rec_all[:, qb:qb + 1],
                                             psB[:, d:d + 1])
                        nc.vector.tensor_scalar(
                            out_sb[:, qb, :], psB[:, 0:d],
                            rec_all[:, qb:qb + 1], None, MULT)
                    # ln(1/sum) for all blocks in one ACT op
                    nc.scalar.activation(out=lnr_all, in_=rec_all, func=LN)

                    # ---- path A: S tiles -> attn --------------------------
                    for qb in range(NQB):
                        qt = (q0 + qb * P) // P
                        attn_sb = attnp.tile([P, s], F32, tag="attn")
                        qs = slice(q0 + qb * P, q0 + (qb + 1) * P)
                        for half_k in range(s // qh):
                            k0 = half_k * qh
                            psA = ps_b.tile([P, qh], F32, tag="bigA")
                            # adjacent k chunks as concurrent row-group
                            # matmuls (d=64 contraction, rows 0-63 / 64-127)
                            for c in range(NQC):
                                lo = (c % 2) * d
                                nc.tensor.matmul(
                                    psA[:, c * QC:(c + 1) * QC],
                                    lhsT=qsb[lo:lo + d, qs],
                                    rhs=ksb[lo:lo + d,
                                            k0 + c * QC:k0 + (c + 1) * QC],
                                    start=True, stop=True)
                            # exp(s - ln sum) = exp(s)/sum
                            nc.scalar.activation(
                                out=attn_sb[:, k0:k0 + qh], in_=psA,
                                func=EXP, bias=lnr_all[:, qb:qb + 1])
                        meng = nc.vector if qb % 4 == 3 else nc.gpsimd
                        meng.tensor_tensor(
                            out=attn_sb, in0=attn_sb, in1=maskN[:, qt, :],
                            op=MULT)
                        nc.sync.dma_start(
                            attn_d[h, q0 + qb * P: q0 + (qb + 1) * P, :],
                            attn_sb)
                    nc.sync.dma_start(out_d[h, half], out_sb)

    nc.compile()
    return nc


def prep_core_inputs(q, k, v, m01_N, m01_T, s=S, d=D):
    """Host-side shard prep for one core.

    q/k/v: [hpc, s, d] float32 for this core's heads.
    m01_N/m01_T: fp8 {0,1} masks in [P, s//P, s] layouts (shared per batch).
    """
    KT = s // P
    scale = np.float32(1.0 / np.sqrt(d))
    hpc = q.shape[0]
    # rows 0:d and d:2d hold the same data — feeds the two concurrent
    # row-group matmuls (base partitions 0 and 64)
    qT = np.empty((hpc, P, s), np.float32)
    qT[:, 0:d, :] = np.transpose(q, (0, 2, 1)) * scale
    qT[:, d:2 * d, :] = qT[:, 0:d, :]
    kT = np.empty((hpc, P, s), np.float32)
    kT[:, 0:d, :] = np.transpose(k, (0, 2, 1))
    kT[:, d:2 * d, :] = kT[:, 0:d, :]
    vt = v.reshape(q.shape[0], KT, P, d).transpose(0, 2, 1, 3)  # [hpc,P,KT,d]
    v1 = np.concatenate(
        [vt, np.ones(vt.shape[:-1] + (1,), np.float32)], axis=-1)
    return {
        "qT": qT,
        "kT": kT,
        "v1": np.ascontiguousarray(v1),
        "maskN": m01_N,
        "maskT": m01_T,
    }


def prep_masks(mask_b, s=S):
    """mask_b: [s, s] int -> ({0,1} fp8 [P, s//P, s] natural, transposed)."""
    m01 = (mask_b != 0).astype(np.float32)
    mN = np.ascontiguousarray(
        m01.reshape(s // P, P, s).transpose(1, 0, 2)).astype(FP8NP)
    mT = np.ascontiguousarray(
        m01.T.reshape(s // P, P, s).transpose(1, 0, 2)).astype(FP8NP)
    return mN, mT


_NC_CACHE = {}


def _get_nc():
    if "nc" not in _NC_CACHE:
        _NC_CACHE["nc"] = build_nc()
    return _NC_CACHE["nc"]


def kernel(query, key, value, mask):
    query = np.asarray(query, dtype=np.float32)
    key = np.asarray(key, dtype=np.float32)
    value = np.asarray(value, dtype=np.float32)
    mask = np.asarray(mask)

    nc = _get_nc()

    masks = [prep_masks(mask[b]) for b in range(B)]
    in_maps = []
    for c in range(N_CORES):
        b, hh = divmod(c, N_CORES // B)
        h0 = hh * HPC
        mN, mT = masks[b]
        in_maps.append(prep_core_inputs(
            query[b, h0:h0 + HPC], key[b, h0:h0 + HPC],
            value[b, h0:h0 + HPC], mN, mT))

    res = run_bass_kernel_spmd(nc, in_maps, list(range(N_CORES))).results

    out = np.empty((B, H, S, D), np.float32)
    attn = np.empty((B, H, S, S), np.float32)
    for c in range(N_CORES):
        b, hh = divmod(c, N_CORES // B)
        h0 = hh * HPC
        o = res[c]["out"]  # [HPC, NH, P, NQB, d]
        out[b, h0:h0 + HPC] = o.transpose(0, 1, 3, 2, 4).reshape(HPC, S, D)
        attn[b, h0:h0 + HPC] = res[c]["attn"]
    return out, attn
